# revision 10
# baseline (speedup 1.0000x reference)
"""Single-head attention (B=4, S=4096, E=2048, d=128) on 8 trn2 cores.

Sharding: core c handles (batch b = c//2, seq half h = c%2). Each core
projects q/k/v for its own 2048-row half; the pair (2b, 2b+1) exchanges
K then V via 2-core AllGathers overlapped with compute. Softmax over
keys is permutation-invariant, so per-core key order (own-first) is fine.

Bias algebra: k-bias shifts every key score of a query by a constant
-> softmax-invariant -> dropped. v-bias adds bv to the output post-
normalization -> added on the host. Only the q-bias is applied on
device (folded into the q PSUM evacuation).

Schedule (per core), designed so ACT's exp stream (the attention-phase
bottleneck, ~73us) starts as early as possible and hides under the
PE's ~107us of matmul work:
  warmup MMs (HAM un-throttle) | x DMAs [128,2048] per e-chunk
  k-proj -> k exchange -> q-proj (+bias)
  scores+exp pass A (qb0, qb1)          <- ACT starts ~22us
  v-proj -> v own transposes -> v exchange
  PV/tree qb0 | scores+exp qb2 | PV qb1 | scores+exp qb3 | PV qb2/3
  peer v transposes, pass B likewise, out DMA per qb.
Denominators: DVE halving tree over the contiguous exp region
[128, 8192], then a ones-column matmul; the 4 query blocks' sums share
one PSUM bank at partition offsets 0/32/64/96 (tile_position), with one
accumulation group spanning pass A and B.
"""

import numpy as np
import ml_dtypes

import concourse.tile as tile
from concourse import bacc, mybir
from concourse.bass_utils import run_bass_kernel_spmd
from concourse.masks import make_identity

N_CORES = 8
B, S, E, D = 4, 4096, 2048, 128
HALF = S // 2  # queries / own keys per core
QB = 512  # query block (PSUM bank width in fp32)
NE = E // 128  # 16 e-chunks
NQB = HALF // QB  # 4 query blocks
SCALE = 1.0 / float(np.sqrt(D))

BF16 = mybir.dt.bfloat16
F32 = mybir.dt.float32
AF = mybir.ActivationFunctionType
GROUPS = [[2 * i, 2 * i + 1] for i in range(N_CORES // 2)]

_CACHE = {}


def _build():
    nc = bacc.Bacc(
        trn_type="TRN2", target_bir_lowering=False, debug=False, num_devices=N_CORES
    )

    x_d = nc.dram_tensor("xt", [E, HALF], BF16, kind="ExternalInput").ap()
    # w packed cg-major: [128, cg(3) * e(16) * 128], cg order (k, q, v)
    w_d = nc.dram_tensor("w", [128, 3 * NE * 128], BF16, kind="ExternalInput").ap()
    bias_d = nc.dram_tensor("bias_q", [D, 1], F32, kind="ExternalInput").ap()
    peer_d = nc.dram_tensor("peer", [1, 1], mybir.dt.uint32, kind="ExternalInput").ap()
    out_d = nc.dram_tensor("out_t", [D, HALF], F32, kind="ExternalOutput").ap()
    sums_d = nc.dram_tensor("sums", [1, HALF], F32, kind="ExternalOutput").ap()

    with tile.TileContext(nc) as tc:
        with (
            tc.tile_pool(name="xt", bufs=16) as xt_pool,
            tc.tile_pool(name="wsb", bufs=1) as w_pool,
            tc.tile_pool(name="persist", bufs=1) as persist,
            tc.tile_pool(name="exp", bufs=2) as exp_pool,
            tc.tile_pool(name="c0", bufs=2) as c0_pool,
            tc.tile_pool(name="c1", bufs=2) as c1_pool,
            tc.tile_pool(name="c2", bufs=2) as c2_pool,
            tc.tile_pool(name="c3", bufs=2) as c3_pool,
            tc.tile_pool(name="dram", bufs=1, space="DRAM") as dram_pool,
            tc.tile_pool(name="ps_main", bufs=3, space="PSUM") as ps_main,
            tc.tile_pool(name="ps_acc", bufs=1, space="PSUM") as ps_acc,
            tc.tile_pool(name="ps_sums", bufs=1, space="PSUM") as ps_sums_pool,
        ):
            # ---- constants / warmup fodder ----
            junk = persist.tile([128, QB], BF16, tag="junk")
            nc.gpsimd.memset(junk[:], 0.0)
            ones_col = persist.tile([128, 1], BF16, tag="ones")
            nc.gpsimd.memset(ones_col[:], 1.0)
            ident = persist.tile([128, 128], BF16, tag="ident")
            make_identity(nc, ident[:])
            bias_sb = persist.tile([D, 1], F32, tag="bias")
            nc.scalar.dma_start(bias_sb[:], bias_d[:])

            # peer slot register (host supplies 1 on even cores, 0 on odd)
            peer_reg = nc.sync.alloc_register("peer_slot")
            nc.sync.reg_load(peer_reg, peer_d[0:1, 0:1])
            peer_val = nc.sync.snap(peer_reg, donate=True, min_val=0, max_val=1)

            # ---- warmup matmuls: keep PE busy so HAM un-throttles before
            # the first real projection MMs arrive ----
            ps_warm = ps_acc.tile([128, QB], F32, tag="ps_acc")
            for _ in range(10):
                nc.tensor.matmul(
                    ps_warm[:], lhsT=junk[:, 0:128], rhs=junk[:], start=True, stop=True
                )

            # ---- w loads: cg-major halves; k first (needed first) ----
            w_sb = w_pool.tile([128, 3 * NE * 128], BF16, tag="w")
            WG = NE * 128  # one cg = 2048 cols

            def w_ap(cg, e):
                return w_sb[:, cg * WG + e * 128 : cg * WG + (e + 1) * 128]

            for half, eng in ((0, nc.sync), (1, nc.scalar)):
                eng.dma_start(
                    w_sb[:, half * WG // 2 : (half + 1) * WG // 2],
                    w_d[:, half * WG // 2 : (half + 1) * WG // 2],
                )

            # ---- x loads: one [128, 2048] tile per e-chunk (both quarters);
            # alternate sync/scalar queues; q/v weight loads interleaved ----
            xt = {}
            for e in range(NE):
                t = xt_pool.tile([128, HALF], BF16, tag="xt")
                eng = nc.sync if e % 2 == 0 else nc.scalar
                eng.dma_start(t[:], x_d[e * 128 : (e + 1) * 128, :])
                xt[e] = t
                if e == 3:  # queue q weights behind the first few x tiles
                    for half in range(2):
                        nc.scalar.dma_start(
                            w_sb[:, WG + half * WG // 2 : WG + (half + 1) * WG // 2],
                            w_d[:, WG + half * WG // 2 : WG + (half + 1) * WG // 2],
                        )
                if e == 7:  # then v weights
                    for half in range(2):
                        nc.scalar.dma_start(
                            w_sb[
                                :, 2 * WG + half * WG // 2 : 2 * WG + (half + 1) * WG // 2
                            ],
                            w_d[:, 2 * WG + half * WG // 2 : 2 * WG + (half + 1) * WG // 2],
                        )

            # ---- persistent activations ----
            qT = persist.tile([D, HALF], BF16, tag="qT")
            k_sb = persist.tile([D, S], BF16, tag="k_sb")  # [own kT | peer kT]
            vT_sb = persist.tile([D, S], BF16, tag="vT_sb")  # [own vT | peer vT]
            v_sb = persist.tile([128, (S // 128) * D], BF16, tag="v_sb")
            sums_stage = persist.tile([128, QB], F32, tag="sums_stage")
            o_stage = persist.tile([D, HALF], F32, tag="o_stage")

            def k_ap(c):  # kT chunk c (d on partitions); own 0-15, peer 16-31
                return k_sb[:, c * 128 : (c + 1) * 128]

            def project(cg, dst_evac):
                """Project one column group over the full 2048-row half.

                One weight load per e-chunk serves 4 row-block matmuls into
                two [128, 1024] PSUM tiles; dst_evac(idx, psum_ap) evacuates.
                """
                ps_a = ps_main.tile([128, 2 * QB], F32, tag="ps_main")
                ps_b = ps_main.tile([128, 2 * QB], F32, tag="ps_main")
                ps = [ps_a, ps_b]
                for e in range(NE):
                    for blk in range(4):
                        nc.tensor.matmul(
                            ps[blk // 2][:, (blk % 2) * QB : (blk % 2 + 1) * QB],
                            lhsT=w_ap(cg, e),
                            rhs=xt[e][:, blk * QB : (blk + 1) * QB],
                            start=(e == 0),
                            stop=(e == NE - 1),
                        )
                for i in range(2):
                    dst_evac(i, ps[i][:])

            # ---- k projection, exchange ----
            project(
                0,
                lambda i, ps: nc.vector.tensor_copy(
                    k_sb[:, i * 2 * QB : (i + 1) * 2 * QB], ps
                ),
            )
            cc_in_k = dram_pool.tile([D, HALF], BF16, tag="cc_in_k")
            cc_out_k = dram_pool.tile([2, D, HALF], BF16, tag="cc_out_k")
            nc.gpsimd.dma_start(cc_in_k[:], k_sb[:, 0:HALF])
            nc.gpsimd.collective_compute(
                "AllGather",
                mybir.AluOpType.bypass,
                replica_groups=GROUPS,
                ins=[cc_in_k.opt()],
                outs=[cc_out_k.opt()],
            )
            nc.sync.dma_start(k_sb[:, HALF:S], cc_out_k[peer_val])

            # ---- q projection (bias folded into DVE evacuation) ----
            project(
                1,
                lambda i, ps: nc.vector.tensor_scalar_add(
                    qT[:, i * 2 * QB : (i + 1) * 2 * QB], ps, bias_sb[:]
                ),
            )

            # ---- attention machinery ----
            exp_regions = {}

            def scores_exp(qb, p):
                """Scores + exp for all 8 k-pairs of pass p, query block qb."""
                ex = exp_pool.tile([128, 16 * QB], BF16, tag="exp")
                exp_regions[(qb, p)] = ex
                q_ap = qT[:, qb * QB : (qb + 1) * QB]
                for kp in range(8):
                    ps = ps_main.tile([128, 2 * QB], F32, tag="ps_main")
                    for half in range(2):
                        nc.tensor.matmul(
                            ps[:, half * QB : (half + 1) * QB],
                            lhsT=k_ap(16 * p + 2 * kp + half),
                            rhs=q_ap,
                            start=True,
                            stop=True,
                        )
                    nc.scalar.activation(
                        ex[:, kp * 2 * QB : (kp + 1) * 2 * QB], ps[:], AF.Exp,
                        scale=SCALE,
                    )

            def v_transpose(c):
                ps_t = ps_main.tile([128, 128], BF16, tag="ps_main")
                nc.tensor.transpose(
                    ps_t[:], vT_sb[:, c * 128 : (c + 1) * 128], ident[:]
                )
                nc.vector.tensor_copy(v_sb[:, c * D : (c + 1) * D], ps_t[:])

            ps_sums = ps_sums_pool.tile([128, QB], F32, tag="ps_sums")

            def pv_tree(qb, p):
                """PV accumulation + denominator tree for pass p, block qb."""
                ex = exp_regions.pop((qb, p))
                ps_o = ps_acc.tile([128, QB], F32, tag="ps_acc")
                for kp in range(8):
                    for half in range(2):
                        c = 16 * p + 2 * kp + half
                        off = kp * 2 * QB + half * QB
                        nc.tensor.matmul(
                            ps_o[:],
                            lhsT=v_sb[:, c * D : (c + 1) * D],
                            rhs=ex[:, off : off + QB],
                            start=(kp == 0 and half == 0),
                            stop=(kp == 7 and half == 1),
                        )
                c0 = c0_pool.tile([128, 8 * QB], BF16, tag="c0")
                nc.vector.tensor_add(c0[:], ex[:, 0 : 8 * QB], ex[:, 8 * QB : 16 * QB])
                c1 = c1_pool.tile([128, 4 * QB], BF16, tag="c1")
                nc.vector.tensor_add(c1[:], c0[:, 0 : 4 * QB], c0[:, 4 * QB : 8 * QB])
                c2 = c2_pool.tile([128, 2 * QB], BF16, tag="c2")
                nc.vector.tensor_add(c2[:], c1[:, 0 : 2 * QB], c1[:, 2 * QB : 4 * QB])
                c3 = c3_pool.tile([128, QB], BF16, tag="c3")
                nc.vector.tensor_add(c3[:], c2[:, 0:QB], c2[:, QB : 2 * QB])
                nc.tensor.matmul(
                    ps_sums[32 * qb : 32 * qb + 1, :],
                    lhsT=ones_col[:],
                    rhs=c3[:],
                    start=(p == 0),
                    stop=(p == 1),
                    tile_position=(0, 32 * qb),
                )
                o_sl = o_stage[:, qb * QB : (qb + 1) * QB]
                if p == 0:
                    nc.vector.tensor_copy(o_sl, ps_o[:])
                else:
                    nc.vector.tensor_add(o_sl, o_sl, ps_o[:])
                    nc.sync.dma_start(out_d[:, qb * QB : (qb + 1) * QB], o_sl)
                    # DVE lanes are partition-locked: stage sums on the same
                    # partition as the PSUM slice, DMA moves it to row 0.
                    nc.vector.tensor_copy(
                        sums_stage[32 * qb : 32 * qb + 1, :],
                        ps_sums[32 * qb : 32 * qb + 1, :],
                    )
                    nc.sync.dma_start(
                        sums_d[0:1, qb * QB : (qb + 1) * QB],
                        sums_stage[32 * qb : 32 * qb + 1, :],
                    )

            # ---- pass A interleave: early scores feed ACT while the PE
            # fills its exp-wait slack with v projection + transposes ----
            scores_exp(0, 0)
            scores_exp(1, 0)
            project(
                2,
                lambda i, ps: nc.vector.tensor_copy(
                    vT_sb[:, i * 2 * QB : (i + 1) * 2 * QB], ps
                ),
            )
            cc_in_v = dram_pool.tile([D, HALF], BF16, tag="cc_in_v")
            cc_out_v = dram_pool.tile([2, D, HALF], BF16, tag="cc_out_v")
            nc.gpsimd.dma_start(cc_in_v[:], vT_sb[:, 0:HALF])
            nc.gpsimd.collective_compute(
                "AllGather",
                mybir.AluOpType.bypass,
                replica_groups=GROUPS,
                ins=[cc_in_v.opt()],
                outs=[cc_out_v.opt()],
            )
            nc.sync.dma_start(vT_sb[:, HALF:S], cc_out_v[peer_val])
            for c in range(16):  # own v chunks
                v_transpose(c)
            pv_tree(0, 0)
            scores_exp(2, 0)
            pv_tree(1, 0)
            scores_exp(3, 0)
            pv_tree(2, 0)
            pv_tree(3, 0)

            # ---- pass B: peer chunks ----
            for c in range(16, 32):
                v_transpose(c)
            for qb in range(NQB):
                scores_exp(qb, 1)
                pv_tree(qb, 1)

    nc.compile()
    return nc


def _prep_inputs(x, W, b):
    """Host-side sharding prep: cast bf16, transpose to xT, pack w cg-major."""
    b_f = np.asarray(b, dtype=np.float32)
    bias_q = np.ascontiguousarray(b_f[0:D].reshape(D, 1))  # q bias column
    # W [E, 3D] -> [128p, cg(3), e(16), 128] with cg order (k, q, v)
    w4 = np.asarray(W).astype(ml_dtypes.bfloat16).reshape(NE, 128, 3, D)
    w_bf = np.ascontiguousarray(
        w4.transpose(1, 2, 0, 3)[:, [1, 0, 2], :, :].reshape(128, 3 * NE * D)
    )
    in_maps = []
    for bb in range(B):
        xt_full = np.ascontiguousarray(
            np.asarray(x[bb]).astype(ml_dtypes.bfloat16).T
        )  # [E, S]
        for h in range(2):
            xc = np.ascontiguousarray(xt_full[:, h * HALF : (h + 1) * HALF])
            peer = np.array([[1 - h]], dtype=np.uint32)
            in_maps.append(
                {"xt": xc, "w": w_bf, "bias_q": bias_q, "peer": peer}
            )
    return in_maps


def _run(in_maps, trace=False, trace_kwargs=None):
    if "nc" not in _CACHE:
        _CACHE["nc"] = _build()
    return run_bass_kernel_spmd(
        _CACHE["nc"],
        in_maps,
        list(range(N_CORES)),
        trace=trace,
        **(trace_kwargs or {}),
    )


def kernel(x, W, b):
    in_maps = _prep_inputs(x, W, b)
    res = None
    for attempt in range(3):
        try:
            res = _run(in_maps)
            break
        except Exception:
            if attempt == 2:
                raise
    bv = np.asarray(b, dtype=np.float32)[2 * D : 3 * D]  # v bias, host-applied
    out = np.empty((B, S, D), dtype=np.float32)
    for c in range(N_CORES):
        bb, h = c // 2, c % 2
        o_t = res.results[c]["out_t"]  # [D, HALF]
        sums = res.results[c]["sums"]  # [1, HALF]
        out[bb, h * HALF : (h + 1) * HALF, :] = (o_t / sums).T + bv
    return out
